# revision 17
# baseline (speedup 1.0000x reference)
"""Trainium2 Bass kernel for the BERT span-pair classifier problem.

Computes, for B=2 batches over a 252x252 span-pair grid:
    h    = relu(Ai[i] + Aj[j] + ind(i,j)*w1c + b1)        # [770] per pair
    out  = h @ W2.T + b2                                   # [36]  per pair
    out  = where(span_mask >= 1, out, 0)
    res  = log_softmax(out over the 63504 pairs)           # per (batch, label)
    return res transposed to [B, 36, L*L]

Strategy (8 NeuronCores, SPMD single program):
  - 504 (batch, row-i) rows distributed over 8 cores in 4 slot segments
    (in-span b0/b1, off-span b0/b1); in-span rows round-robin so the only
    runtime-varying offset is `static + core_id`.
  - h produced in [hid-chunk(128) x j] layout by fused tensor_scalar /
    activation(Relu, bias) ops; span indicator realized by a windowed
    overwrite from BjE = Bj0 + w1c * [j <= e].
  - 770->36 matmul streams h against stationary W2^T chunks; b2 and a -BIG
    invalid-pair offset are folded in as extra contraction rows.
  - exp + per-tile softmax sums from ACT activation(Exp, accum_out=..).
  - global log-sum-exp via a 3-round XOR butterfly of remote SBUF DMAs
    (remote_dma_broadcast) instead of a NCCL AllReduce: ~3-6us vs ~36us.
  - final out = mask*logits - lse in bf16, split across DVE/ACT, DMA'd out
    over two queues; host casts to f32 at unshard time.
"""

import math
import os
from contextlib import ExitStack

import numpy as np

import concourse.bass as bass
import concourse.bacc as bacc
import concourse.tile as tile
from concourse import mybir
from concourse._compat import with_exitstack
from concourse.bass_utils import run_bass_kernel_spmd

L = 252
HID = 768
MLP = 770
NLAB = 36
B = 2
NC = 8
KC = 6           # full 128-row hid chunks (6*128 = 768)
BIGNEG = -30.0   # makes exp(invalid pair) ~ 0 while staying fp32-exact

FP32 = mybir.dt.float32
BF16 = mybir.dt.bfloat16
AF = mybir.ActivationFunctionType
ALU = mybir.AluOpType


def _ceil2(x):
    return x + (x & 1)


def plan_slots(spans):
    """Compute the slot layout shared by host + device."""
    segs = []
    slot = 0
    for b in range(B):
        s, e = spans[b]
        n = e - s + 1
        nsl = _ceil2(math.ceil(n / NC))
        segs.append(dict(kind="in", b=b, start=slot, nslots=nsl, s=s, e=e, count=n))
        slot += nsl
    for b in range(B):
        s, e = spans[b]
        rows = [r for r in range(L) if r < s or r > e]
        nsl = _ceil2(math.ceil(len(rows) / NC))
        segs.append(dict(kind="off", b=b, start=slot, nslots=nsl, rows=rows,
                         count=len(rows)))
        slot += nsl
    nslot = slot
    assert nslot % 2 == 0
    return segs, nslot


def slot_map_for_core(segs, nslot, c):
    """-> list over slots of (batch, global_row) or None for padding."""
    m = [None] * nslot
    for sg in segs:
        for k in range(sg["nslots"]):
            idx = NC * k + c
            p = sg["start"] + k
            if idx < sg["count"]:
                if sg["kind"] == "in":
                    m[p] = (sg["b"], sg["s"] + idx)
                else:
                    m[p] = (sg["b"], sg["rows"][idx])
    return m


# ---- engine per-op costs (ns), measured on HW (mb.py) ----
DVE_SEGA = 273.0        # TS bf16 FD=256, scalar-AP, add+max
ACT_SEGA = 491.0        # ACT Relu bias FD=256
DVE_TT = 673.0          # TT psum x bf16 -> bf16, FD=504
ACT_EXP = 700.0         # exp FD=504 + accum read


def act_segb_cost(w):
    return (224 + w) / 1.2 + 120.0   # + dynamic-offset reg writes


def build_kernel(spans, segs, nslot, use_cc, handles):
    ntile = nslot // 2
    slot_batch = []
    slot_seg = []
    for sg in segs:
        for k in range(sg["nslots"]):
            slot_batch.append(sg["b"])
            slot_seg.append(sg)
    btiles = {b: [t for t in range(ntile)
                  if slot_batch[2 * t] == b] for b in range(B)}
    for b in range(B):
        ts_ = btiles[b]
        assert all(slot_batch[2 * t] == slot_batch[2 * t + 1] for t in ts_)

    HW = 768           # h tile width: 2 slots x SLOTW
    SLOTW = 384        # per-slot region inside an h tile (252 real + pad)

    @with_exitstack
    def kern(ctx: ExitStack, tc: tile.TileContext, outs, ins):
        nc = tc.nc
        w1iT = ins["w1iT"]      # [768, 770] bf16 (DRAM)
        w1jT = ins["w1jT"]      # [768, 770] bf16
        w1c = ins["w1c"]        # [770] f32
        b1 = ins["b1"]          # [770] f32
        w2T6 = ins["w2T6"]      # [768, 36] bf16  (chunks 0..5 of W2^T)
        w2Tt = ins["w2Tt"]      # [3, 36] bf16    (rows 768, 769, b2)
        vecsT = ins["vecsT"]    # [768, 504] bf16  (both batches, all j)
        myvT = ins["myvT"]      # [768, nslot] bf16 (per-core slot rows)
        maskb = ins["maskb"]    # [1, nslot*384] bf16: the 0/1 mask row
        maskrep = ins["maskrep"]  # [nslot*252] bf16 flat mask (s_out layout)
        e2f = ins["e2f"]        # [128, 2] f32: col b = 1 if this core owns row s_b
        outd = outs["out"]      # [36, nslot*252] bf16
        lseo = outs["lse"]      # [36, 2] f32

        fp = ctx.enter_context(tc.tile_pool(name="fp", bufs=1))
        prep_ps = ctx.enter_context(tc.tile_pool(name="prep_ps", bufs=2, space="PSUM"))
        main_ps = ctx.enter_context(tc.tile_pool(name="main_ps", bufs=3, space="PSUM"))
        hp = [ctx.enter_context(tc.tile_pool(name=f"h{c}", bufs=3)) for c in range(7)]
        esp = ctx.enter_context(tc.tile_pool(name="esp", bufs=2))
        dram = ctx.enter_context(tc.tile_pool(name="dram", bufs=1, space="DRAM"))

        # ---- persistent SBUF ----
        s_w1i = [fp.tile([128, MLP], BF16, tag=f"w1i{k}", name=f"w1i{k}") for k in range(KC)]
        s_w1j = [fp.tile([128, MLP], BF16, tag=f"w1j{k}", name=f"w1j{k}") for k in range(KC)]
        s_vT = [fp.tile([128, 2 * L], BF16, tag=f"vT{k}", name=f"vT{k}") for k in range(KC)]
        s_myv = [fp.tile([128, nslot], BF16, tag=f"myv{k}", name=f"myv{k}") for k in range(KC)]
        s_w2 = fp.tile([128, 6 * NLAB], BF16)      # W2T chunks 0..5 side by side
        s_w2t4 = fp.tile([3, NLAB], BF16)          # [W2T768, W2T769, b2]
        s_w1c = fp.tile([128, 7], FP32)            # w1c per-chunk columns
        s_b1 = fp.tile([128, 7], FP32)
        s_b1w = fp.tile([128, 7], FP32)
        s_e2f = fp.tile([128, 2], FP32)
        CS = [128] * KC + [2]                      # chunk sizes of 770
        s_bj0 = [fp.tile([128, B * 512], BF16, tag=f"bj0_{c}", name=f"bj0_{c}") for c in range(7)]
        s_bje = [fp.tile([128, B * 512], BF16, tag=f"bje_{c}", name=f"bje_{c}") for c in range(7)]
        s_ai = [fp.tile([128, nslot], FP32, tag=f"ai{c}", name=f"ai{c}") for c in range(7)]
        s_aiE2 = [fp.tile([128, B], FP32, tag=f"aiE2{c}", name=f"aiE2{c}") for c in range(7)]
        s_mrep = fp.tile([NLAB, nslot * L], BF16)  # mask replicated over labels
        s_acc = fp.tile([NLAB, ntile], FP32)
        s_out = handles["s_out"]
        s_x = handles["s_x"]
        s_g = handles["s_g"]
        s_nlse = handles["s_nlse"]

        # ---- load constants (spread across HWDGE queues) ----
        dmae = [nc.sync, nc.scalar]
        for k in range(KC):
            dmae[k % 2].dma_start(out=s_w1i[k], in_=w1iT[128 * k:128 * (k + 1), :])
            dmae[(k + 1) % 2].dma_start(out=s_w1j[k],
                                        in_=w1jT[128 * k:128 * (k + 1), :])
            dmae[k % 2].dma_start(out=s_vT[k],
                                  in_=vecsT[128 * k:128 * (k + 1), :])
            dmae[(k + 1) % 2].dma_start(out=s_myv[k],
                                        in_=myvT[128 * k:128 * (k + 1), :])
            dmae[k % 2].dma_start(out=s_w2[:, NLAB * k:NLAB * (k + 1)],
                                  in_=w2T6[128 * k:128 * (k + 1), :])
        nc.sync.dma_start(out=s_w2t4, in_=w2Tt)
        nc.vector.memset(s_w1c, 0.0)
        nc.vector.memset(s_b1, 0.0)
        w1c2 = w1c[0:HID].rearrange("(c p) -> p c", p=128)
        nc.sync.dma_start(out=s_w1c[:, 0:KC], in_=w1c2)
        nc.sync.dma_start(out=s_w1c[0:2, KC:7], in_=w1c[HID:MLP].rearrange("(p o) -> p o", o=1))
        b12 = b1[0:HID].rearrange("(c p) -> p c", p=128)
        nc.sync.dma_start(out=s_b1[:, 0:KC], in_=b12)
        nc.sync.dma_start(out=s_b1[0:2, KC:7], in_=b1[HID:MLP].rearrange("(p o) -> p o", o=1))
        nc.sync.dma_start(out=s_e2f, in_=e2f)
        nc.vector.tensor_tensor(out=s_b1w, in0=s_b1, in1=s_w1c, op=ALU.add)
        # replicated mask (one broadcast DMA per quarter to spread queues)
        Q = nslot * L // 4
        for q in range(4):
            src = maskrep[q * Q:(q + 1) * Q]
            nc.gpsimd.dma_start(
                out=s_mrep[:, q * Q:(q + 1) * Q],
                in_=bass.AP(tensor=src.tensor, offset=src.offset,
                            ap=[[0, NLAB], [1, Q]]))

        # ---- prep: AjT -> Bj0/BjE, AiT ----
        for c in range(7):
            cs = CS[c]
            mlo = 128 * c
            psA = prep_ps.tile([128, 2 * L], FP32, tag="psA", name=f"psA{c}")
            for k in range(KC):
                nc.tensor.matmul(psA[:cs, :], s_w1j[k][:, mlo:mlo + cs], s_vT[k],
                                 start=(k == 0), stop=(k == KC - 1))
            bje = s_bje[c]
            for b in range(B):
                e = spans[b][1]
                nc.vector.tensor_scalar(
                    out=s_bj0[c][:cs, 512 * b:512 * b + L],
                    in0=psA[:cs, L * b:L * (b + 1)],
                    scalar1=s_b1[:cs, c:c + 1], scalar2=None, op0=ALU.add)
                nc.gpsimd.memset(s_bj0[c][:cs, 512 * b + L:512 * (b + 1)], 0.0)
                nc.vector.tensor_scalar(
                    out=bje[:cs, 512 * b:512 * b + e + 1],
                    in0=psA[:cs, L * b:L * b + e + 1],
                    scalar1=s_b1w[:cs, c:c + 1], scalar2=None, op0=ALU.add)
                if e + 1 < L:
                    nc.vector.tensor_scalar(
                        out=bje[:cs, 512 * b + e + 1:512 * b + L],
                        in0=psA[:cs, L * b + e + 1:L * (b + 1)],
                        scalar1=s_b1[:cs, c:c + 1], scalar2=None, op0=ALU.add)
                nc.vector.memset(bje[:cs, 512 * b + L:512 * (b + 1)], 0.0)
            psI = prep_ps.tile([128, nslot], FP32, tag="psI", name=f"psI{c}")
            for k in range(KC):
                nc.tensor.matmul(psI[:cs, :], s_w1i[k][:, mlo:mlo + cs], s_myv[k],
                                 start=(k == 0), stop=(k == KC - 1))
            nc.vector.tensor_copy(out=s_ai[c][:cs, :], in_=psI[:cs, :])
            for b in range(B):
                p_e2 = next(sg for sg in segs
                            if sg["kind"] == "in" and sg["b"] == b)["start"]
                nc.vector.tensor_tensor(
                    out=s_aiE2[c][:cs, b:b + 1],
                    in0=s_w1c[:cs, c:c + 1], in1=s_e2f[:cs, b:b + 1],
                    op=ALU.mult)
                nc.vector.tensor_tensor(
                    out=s_aiE2[c][:cs, b:b + 1],
                    in0=s_aiE2[c][:cs, b:b + 1], in1=s_ai[c][:cs, p_e2:p_e2 + 1],
                    op=ALU.add)

        nc.vector.memset(s_x[0], 0.0)

        pid = {}

        def eng_pid(eng):
            if eng not in pid:
                pid[eng] = eng.partition_id()
            return pid[eng]

        def ts_relu(eng, out, in0, sc):
            if eng is nc.scalar:
                nc.scalar.activation(out, in0, AF.Relu, bias=sc, scale=1.0)
            else:
                eng.tensor_scalar(out=out, in0=in0, scalar1=sc, scalar2=0.0,
                                  op0=ALU.add, op1=ALU.max)

        # ---- main loop over 2-slot tiles ----
        for t in range(ntile):
            hts = [hp[c].tile([128 if c < 6 else 3, HW], BF16, tag=f"ht{c}",
                              name=f"ht{c}_{t}") for c in range(7)]
            nc.sync.dma_start(out=hts[6][2:3, :], in_=maskb[:, HW * t:HW * (t + 1)])
            # greedy DVE/ACT balance for this tile
            dve_load = DVE_TT
            act_load = ACT_EXP
            emits = []                # (which, args...)
            for c in range(7):
                cs = CS[c]
                for sl in range(2):
                    p = 2 * t + sl
                    b = slot_batch[p]
                    sg = slot_seg[p]
                    o_t, o_base = hts[c], SLOTW * sl
                    # segA
                    ca, cb = DVE_SEGA, ACT_SEGA
                    if sg["kind"] == "in":
                        kk = p - sg["start"]
                        Wd = max(1, sg["e"] - sg["s"] - NC * kk + 1)
                        act_load += act_segb_cost(Wd)
                        emits.append(("B", c, cs, o_t, o_base, p, b, sg, kk, Wd))
                    if dve_load + ca <= act_load + cb:
                        dve_load += ca
                        emits.append(("A", nc.vector, c, cs, o_t, o_base, p, b))
                    else:
                        act_load += cb
                        emits.append(("A", nc.scalar, c, cs, o_t, o_base, p, b))
                    if sg["kind"] == "in" and p == sg["start"]:
                        emits.append(("E", c, cs, o_t, o_base, b, sg))
            for em in emits:
                if em[0] == "A":
                    _, eng, c, cs, o_t, o_base, p, b = em
                    ts_relu(eng, o_t[:cs, o_base:o_base + 256],
                            s_bj0[c][:cs, 512 * b:512 * b + 256],
                            s_ai[c][:cs, p:p + 1])
                elif em[0] == "B":
                    _, c, cs, o_t, o_base, p, b, sg, kk, Wd = em
                    ioff = sg["s"] + NC * kk + eng_pid(nc.scalar)
                    ts_relu(nc.scalar, o_t[:cs, bass.ds(o_base + ioff, Wd)],
                            s_bje[c][:cs, bass.ds(512 * b + ioff, Wd)],
                            s_ai[c][:cs, p:p + 1])
                else:
                    _, c, cs, o_t, o_base, b, sg = em
                    e = sg["e"]
                    ts_relu(nc.vector, o_t[:cs, o_base + e:o_base + e + 1],
                            s_bje[c][:cs, 512 * b + e:512 * b + e + 1],
                            s_aiE2[c][:cs, b:b + 1])

            # matmul: psum[36, 504] over 6 full chunks + tail (h768/769, m*b2)
            ps = main_ps.tile([NLAB, 2 * L], FP32, tag="ps", name=f"ps{t}")
            rhs6 = [hts[c][:, :].rearrange("p (s w) -> p s w", w=SLOTW)
                    [:, :, 0:L] for c in range(6)]
            for c in range(6):
                nc.tensor.matmul(ps, s_w2[:, NLAB * c:NLAB * (c + 1)], rhs6[c],
                                 start=(c == 0), stop=False)
            rhs_t = hts[6][:, :].rearrange("p (s w) -> p s w", w=SLOTW)[:, :, 0:L]
            nc.tensor.matmul(ps, s_w2t4, rhs_t, start=False, stop=True)

            # masked logits -> bf16 out buffer
            nc.vector.tensor_tensor(out=s_out[:, 2 * L * t:2 * L * (t + 1)],
                                    in0=ps, in1=s_mrep[:, 2 * L * t:2 * L * (t + 1)],
                                    op=ALU.mult)
            # exp + per-tile softmax sum (invalid pairs contribute exp(0)=1,
            # exactly matching the reference where masked logits are 0)
            esc = esp.tile([NLAB, 2 * L], BF16, tag="esc", name=f"esc{t}")
            nc.scalar.activation(esc, s_out[:, 2 * L * t:2 * L * (t + 1)],
                                 AF.Exp, accum_out=s_acc[:, t:t + 1])

        # ---- LSE: tile sums + cross-core exchange + log ----
        for b in range(B):
            rs = btiles[b]
            runs = []
            st = rs[0]
            for a, bb in zip(rs, rs[1:] + [None]):
                if bb != a + 1:
                    runs.append((st, a))
                    st = bb
            acc_parts = []
            for (u0, u1) in runs:
                tmp = fp.tile([NLAB, 1], FP32, tag=f"tr{b}_{u0}", name=f"tr{b}_{u0}")
                nc.vector.tensor_reduce(out=tmp, in_=s_acc[:, u0:u1 + 1],
                                        axis=mybir.AxisListType.X, op=ALU.add)
                acc_parts.append(tmp)
            for tmp in acc_parts:
                nc.vector.tensor_tensor(out=s_x[0][0:NLAB, b:b + 1],
                                        in0=s_x[0][0:NLAB, b:b + 1], in1=tmp,
                                        op=ALU.add)

        if use_cc:
            cc_in = dram.tile([128, B], FP32, name="cc_in")
            cc_out = nc.dram_tensor("cc_out", [128, B], FP32, kind="Internal",
                                    addr_space="Shared").ap()
            nc.sync.dma_start(out=cc_in, in_=s_x[0])
            nc.gpsimd.collective_compute(
                "AllReduce", ALU.add, replica_groups=[list(range(NC))],
                ins=[cc_in[:]], outs=[cc_out], cc_dim="Partition")
            nc.sync.dma_start(out=s_x[3], in_=cc_out)

    return kern


def tail_groups(slot_batch, ntile):
    groups = []
    g0 = 0
    while g0 < ntile:
        if g0 + 1 < ntile and slot_batch[2 * (g0 + 1)] == slot_batch[2 * g0]:
            groups.append((g0, 2))
            g0 += 2
        else:
            groups.append((g0, 1))
            g0 += 1
    return groups


def build_post(nc, H, outd, lseo, slot_batch, ntile, use_cc):
    """Raw-bass epilogue: butterfly all-reduce of the exp sums, -log(lse),
    tail adds and output DMAs.  Engine FIFO order + explicit semaphores."""
    gp, dv, ac, sy = nc.gpsimd, nc.vector, nc.scalar, nc.sync
    s_x, s_g, s_out, s_nlse = H["s_x"], H["s_g"], H["s_out"], H["s_nlse"]

    nc.all_engine_barrier()
    sems = H["sems"]
    nsem, dsem, asem, osem, xsem = (sems["nsem"], sems["dsem"], sems["asem"],
                                    sems["osem"], sems["xsem"])
    gsem = sems["gsem"]
    if not use_cc:
        lsem, psem = sems["lsem"], sems["psem"]
        rsem = sems["rsem"]
        gp.bir_kernel_barrier_wait([list(range(NC))])
        for r in range(3):
            if r > 0:
                gp.wait_ge(xsem, r)
            prep = gp.remote_dma_broadcast(
                out_ap=s_g[r][:, :], in_ap=s_x[r][:, :],
                remote_sem=rsem[r], local_sem=lsem,
                rdests=[(0, 1 << r) if k == (1 << r) else None
                        for k in range(8)])
            prep.then_inc(psem, 1)
            gp.wait_ge(psem, r + 1)
            gp.trigger_dma(count=1)
            dv.wait_ge(rsem[r], 2)
            dv.tensor_tensor(out=s_x[r + 1], in0=s_x[r], in1=s_g[r],
                             op=ALU.add).then_inc(xsem, 1)
    else:
        dv.tensor_copy(out=s_x[3], in_=s_x[3]).then_inc(xsem, 3)

    ac.wait_ge(xsem, 3)
    ac.activation(s_nlse, s_x[3][0:NLAB, :], AF.Ln).then_inc(nsem, 1)
    dv.wait_ge(nsem, 1)
    dv.tensor_scalar(out=s_nlse, in0=s_nlse, scalar1=-1.0,
                     scalar2=None, op0=ALU.mult).then_inc(nsem, 1)
    dv.wait_ge(nsem, 2)
    ac.wait_ge(nsem, 2)
    sy.wait_ge(xsem, 3)
    sy.dma_start(out=lseo, in_=s_x[3][0:NLAB, :]).then_inc(osem, 16)

    groups = tail_groups(slot_batch, ntile)
    kd = ka = nout = 0
    for gi, (t0, gn) in enumerate(groups):
        b = slot_batch[2 * t0]
        seg = s_out[:, 2 * L * t0:2 * L * (t0 + gn)]
        oseg = outd[:, 2 * L * t0:2 * L * (t0 + gn)]
        if gi % 2 == 0:
            dv.tensor_scalar(out=seg, in0=seg, scalar1=s_nlse[:, b:b + 1],
                             scalar2=None, op0=ALU.add).then_inc(dsem, 1)
            kd += 1
            sy.wait_ge(dsem, kd)
            sy.dma_start(out=oseg, in_=seg).then_inc(osem, 16)
        else:
            ac.activation(seg, seg, AF.Identity,
                          bias=s_nlse[:, b:b + 1], scale=1.0).then_inc(asem, 1)
            ka += 1
            gp.wait_ge(asem, ka)
            gp.dma_start(out=oseg, in_=seg).then_inc(gsem, 16)
        nout += 1
    gp.wait_ge(osem, 16 * (nout - ka + 1))
    gp.wait_ge(gsem, 16 * ka)


def kernel(**inputs) -> np.ndarray:
    hidden = np.asarray(inputs["hidden"], dtype=np.float32)
    pred_spans = np.asarray(inputs["pred_spans"]).astype(np.int64)
    span_mask = np.asarray(inputs["span_mask"]).astype(np.int32)
    W1 = np.asarray(inputs["W1"], dtype=np.float32)
    b1 = np.asarray(inputs["b1"], dtype=np.float32)
    W2 = np.asarray(inputs["W2"], dtype=np.float32)
    b2 = np.asarray(inputs["b2"], dtype=np.float32)

    spans = [(int(pred_spans[b, 0]), int(pred_spans[b, 1])) for b in range(B)]
    segs, nslot = plan_slots(spans)
    ntile = nslot // 2

    vecs = hidden[:, 1:L + 1, :]                       # [B, L, 768]
    vecsT = np.concatenate([vecs[0].T, vecs[1].T], axis=1)   # [768, 504]
    W1T = W1.T                                          # [1537, 770]
    w1iT = np.ascontiguousarray(W1T[0:HID]).astype(np.float32)
    w1jT = np.ascontiguousarray(W1T[HID:2 * HID]).astype(np.float32)
    w1c = np.ascontiguousarray(W1T[2 * HID]).astype(np.float32)
    W2T = np.ascontiguousarray(W2.T)                    # [770, 36]
    w2T6 = W2T[0:HID]
    w2Tt = np.stack([W2T[768], W2T[769], b2], axis=0)

    maskf_full = span_mask.astype(np.float32).clip(0, 1)

    in_maps = []
    slot_maps = []
    for c in range(NC):
        sm = slot_map_for_core(segs, nslot, c)
        slot_maps.append(sm)
        myv = np.zeros((HID, nslot), np.float32)
        maskf = np.zeros((nslot, L), np.float32)
        flags = np.zeros((nslot, 2), np.float32)
        for p, ent in enumerate(sm):
            if ent is None:
                continue
            b, r = ent
            myv[:, p] = vecs[b, r]
            maskf[p] = maskf_full[r]
            flags[p, b] = 1.0
        maskb = np.zeros((1, nslot * 384), np.float32)
        for t in range(ntile):
            for sl in range(2):
                p = 2 * t + sl
                o = 768 * t + 384 * sl
                maskb[0, o:o + L] = maskf[p]
        maskrep = maskf.reshape(-1)                     # [nslot*252]
        e2f = np.zeros((128, 2), np.float32)
        for b in range(B):
            if c == 0:
                e2f[:, b] = 1.0      # row s_b lives on core 0 (slot seg start)
        in_maps.append({
            "w1iT": w1iT, "w1jT": w1jT, "w1c": w1c, "b1": b1,
            "w2T6": w2T6, "w2Tt": w2Tt,
            "vecsT": vecsT, "myvT": myv,
            "maskb": maskb, "maskrep": maskrep, "e2f": e2f,
        })

    # ---- build program ----
    nc = bacc.Bacc("TRN2", target_bir_lowering=False, debug=False,
                   enable_asserts=False, num_devices=NC)

    def mk(name, arr, dt):
        return nc.dram_tensor(name, list(arr.shape), dt, kind="ExternalInput").ap()

    ml_bf = lambda n, a: mk(n, a, BF16)
    ml_f32 = lambda n, a: mk(n, a, FP32)
    ex = in_maps[0]
    ins_aps = {
        "w1iT": ml_bf("w1iT", ex["w1iT"]), "w1jT": ml_bf("w1jT", ex["w1jT"]),
        "w1c": ml_f32("w1c", ex["w1c"]), "b1": ml_f32("b1", ex["b1"]),
        "w2T6": ml_bf("w2T6", ex["w2T6"]), "w2Tt": ml_bf("w2Tt", ex["w2Tt"]),
        "vecsT": ml_bf("vecsT", ex["vecsT"]), "myvT": ml_bf("myvT", ex["myvT"]),
        "maskb": ml_bf("maskb", ex["maskb"]),
        "maskrep": ml_bf("maskrep", ex["maskrep"]),
        "e2f": ml_f32("e2f", ex["e2f"]),
    }
    outs_aps = {
        "out": nc.dram_tensor("out", [NLAB, nslot * L], BF16,
                              kind="ExternalOutput").ap(),
        "lse": nc.dram_tensor("lse", [NLAB, 2], FP32,
                              kind="ExternalOutput").ap(),
    }

    use_cc = bool(int(os.environ.get("BK_CC", "1")))
    handles = {
        "s_out": nc.alloc_sbuf_tensor("s_out", [NLAB, nslot * L], BF16).ap(),
        "s_x": [nc.alloc_sbuf_tensor(f"s_x{r}", [128, B], FP32).ap()
                for r in range(4)],
        "s_g": [nc.alloc_sbuf_tensor(f"s_g{r}", [128, B], FP32).ap()
                for r in range(3)],
        "s_nlse": nc.alloc_sbuf_tensor("s_nlse", [NLAB, B], FP32).ap(),
        "sems": {
            "nsem": nc.alloc_semaphore("nsem"),
            "dsem": nc.alloc_semaphore("dsem"),
            "asem": nc.alloc_semaphore("asem"),
            "osem": nc.alloc_semaphore("osem"),
            "xsem": nc.alloc_semaphore("xsem"),
            "lsem": nc.alloc_semaphore("rdma_lsem"),
            "psem": nc.alloc_semaphore("rdma_psem"),
            "gsem": nc.alloc_semaphore("gsem"),
            "rsem": [nc.alloc_semaphore(f"rdma_rsem{r}") for r in range(3)],
        },
    }
    kern = build_kernel(spans, segs, nslot, use_cc, handles)
    with tile.TileContext(nc) as t:
        kern(t, outs_aps, ins_aps)
    slot_batch = []
    for sg in segs:
        for k in range(sg["nslots"]):
            slot_batch.append(sg["b"])
    build_post(nc, handles, outs_aps["out"], outs_aps["lse"],
               slot_batch, ntile, use_cc)
    nc.compile()

    def cast_maps(m):
        out = {}
        for k, v in m.items():
            dt = ins_aps[k].dtype
            if dt == BF16:
                out[k] = v.astype(mybir.dt.np(BF16))
            else:
                out[k] = v.astype(np.float32)
        return out

    in_maps_c = [cast_maps(m) for m in in_maps]

    if os.environ.get("BK_BUILD_ONLY"):
        print("BUILD OK")
        return np.zeros((B, NLAB, L * L), np.float32)

    if os.environ.get("BK_SIM"):
        import concourse.bass_interp as bi
        from concourse.bass_interp import MultiCoreSim

        # fake-nrt container: no real device maps; 8 cores = device 0, nc 0-7
        import concourse.libnrt as lnrt
        lnrt.get_trn2_nc_mapping = lambda: {(0, i): i for i in range(8)}
        _rmap = lambda: {0: 0}
        lnrt.get_device_id_to_routing_id_mapping = _rmap
        bi.pnc_id_to_device_and_real_nc_index = lambda cid: (0, cid % 8)
        bi.get_device_id_to_routing_id_mapping = _rmap
        bi.nc_to_real_nc = lambda dev, nc: nc

        nc.m.detect_race_conditions = False
        sim = MultiCoreSim(nc, num_cores=NC, require_finite=False,
                           require_nnan=False)
        for c, cs in sim.cores.items():
            for name, arr in in_maps_c[c].items():
                cs.tensor(name)[:] = arr
            if nc.partition_id_tensor is not None:
                cs.tensor(nc.partition_id_tensor.name)[:] = np.array(
                    [[c]], dtype=np.uint32)
        sim.simulate(check_with_hw=False)

        class _R:
            results = [{"out": np.asarray(sim.cores[c].tensor("out")),
                        "lse": np.asarray(sim.cores[c].tensor("lse"))}
                       for c in range(NC)]
        res = _R()
    else:
        trace = bool(int(os.environ.get("BK_TRACE", "0")))
        res = run_bass_kernel_spmd(nc, in_maps_c, core_ids=list(range(NC)),
                                   trace=trace)
        if trace and res.exec_time_ns is not None:
            print(f"HW exec time: {res.exec_time_ns} ns")

    if os.environ.get("BK_DBG_LSE"):
        for c in range(NC):
            print(f"core{c} lse[0:3,:]:", np.asarray(res.results[c]["lse"])[0:3].tolist())
    # ---- unshard ----
    out_full = np.zeros((B, NLAB, L * L), np.float32)
    for c in range(NC):
        oc = np.asarray(res.results[c]["out"]).astype(np.float32)
        for p, ent in enumerate(slot_maps[c]):
            if ent is None:
                continue
            b, r = ent
            out_full[b, :, L * r:L * (r + 1)] = oc[:, L * p:L * (p + 1)]
    return out_full


# revision 18
# speedup vs baseline: 1.0070x; 1.0070x over previous
"""Trainium2 Bass kernel for the BERT span-pair classifier problem.

Computes, for B=2 batches over a 252x252 span-pair grid:
    h    = relu(Ai[i] + Aj[j] + ind(i,j)*w1c + b1)        # [770] per pair
    out  = h @ W2.T + b2                                   # [36]  per pair
    out  = where(span_mask >= 1, out, 0)
    res  = log_softmax(out over the 63504 pairs)           # per (batch, label)
    return res transposed to [B, 36, L*L]

Strategy (8 NeuronCores, SPMD single program):
  - 504 (batch, row-i) rows distributed over 8 cores in 4 slot segments
    (in-span b0/b1, off-span b0/b1); in-span rows round-robin so the only
    runtime-varying offset is `static + core_id`.
  - h produced in [hid-chunk(128) x j] layout by fused tensor_scalar /
    activation(Relu, bias) ops; span indicator realized by a windowed
    overwrite from BjE = Bj0 + w1c * [j <= e].
  - 770->36 matmul streams h against stationary W2^T chunks; b2 and a -BIG
    invalid-pair offset are folded in as extra contraction rows.
  - exp + per-tile softmax sums from ACT activation(Exp, accum_out=..).
  - global log-sum-exp via a 3-round XOR butterfly of remote SBUF DMAs
    (remote_dma_broadcast) instead of a NCCL AllReduce: ~3-6us vs ~36us.
  - final out = mask*logits - lse in bf16, split across DVE/ACT, DMA'd out
    over two queues; host casts to f32 at unshard time.
"""

import math
import os
from contextlib import ExitStack

import numpy as np

import concourse.bass as bass
import concourse.bacc as bacc
import concourse.tile as tile
from concourse import mybir
from concourse._compat import with_exitstack
from concourse.bass_utils import run_bass_kernel_spmd

L = 252
HID = 768
MLP = 770
NLAB = 36
B = 2
NC = 8
KC = 6           # full 128-row hid chunks (6*128 = 768)
BIGNEG = -30.0   # makes exp(invalid pair) ~ 0 while staying fp32-exact

FP32 = mybir.dt.float32
BF16 = mybir.dt.bfloat16
AF = mybir.ActivationFunctionType
ALU = mybir.AluOpType


def _ceil2(x):
    return x + (x & 1)


def plan_slots(spans):
    """Compute the slot layout shared by host + device."""
    segs = []
    slot = 0
    for b in range(B):
        s, e = spans[b]
        n = e - s + 1
        nsl = _ceil2(math.ceil(n / NC))
        segs.append(dict(kind="in", b=b, start=slot, nslots=nsl, s=s, e=e, count=n))
        slot += nsl
    for b in range(B):
        s, e = spans[b]
        rows = [r for r in range(L) if r < s or r > e]
        nsl = _ceil2(math.ceil(len(rows) / NC))
        segs.append(dict(kind="off", b=b, start=slot, nslots=nsl, rows=rows,
                         count=len(rows)))
        slot += nsl
    nslot = slot
    assert nslot % 2 == 0
    return segs, nslot


def slot_map_for_core(segs, nslot, c):
    """-> list over slots of (batch, global_row) or None for padding."""
    m = [None] * nslot
    for sg in segs:
        for k in range(sg["nslots"]):
            idx = NC * k + c
            p = sg["start"] + k
            if idx < sg["count"]:
                if sg["kind"] == "in":
                    m[p] = (sg["b"], sg["s"] + idx)
                else:
                    m[p] = (sg["b"], sg["rows"][idx])
    return m


# ---- engine per-op costs (ns), measured on HW (mb.py) ----
DVE_SEGA = 273.0        # TS bf16 FD=256, scalar-AP, add+max
ACT_SEGA = 491.0        # ACT Relu bias FD=256
DVE_TT = 673.0          # TT psum x bf16 -> bf16, FD=504
ACT_EXP = 700.0         # exp FD=504 + accum read


def act_segb_cost(w):
    return (224 + w) / 1.2 + 120.0   # + dynamic-offset reg writes


def build_kernel(spans, segs, nslot, use_cc, handles):
    ntile = nslot // 2
    slot_batch = []
    slot_seg = []
    for sg in segs:
        for k in range(sg["nslots"]):
            slot_batch.append(sg["b"])
            slot_seg.append(sg)
    btiles = {b: [t for t in range(ntile)
                  if slot_batch[2 * t] == b] for b in range(B)}
    for b in range(B):
        ts_ = btiles[b]
        assert all(slot_batch[2 * t] == slot_batch[2 * t + 1] for t in ts_)

    HW = 768           # h tile width: 2 slots x SLOTW
    SLOTW = 384        # per-slot region inside an h tile (252 real + pad)

    @with_exitstack
    def kern(ctx: ExitStack, tc: tile.TileContext, outs, ins):
        nc = tc.nc
        w1iT = ins["w1iT"]      # [768, 770] bf16 (DRAM)
        w1jT = ins["w1jT"]      # [768, 770] bf16
        w1c = ins["w1c"]        # [770] f32
        b1 = ins["b1"]          # [770] f32
        w2T6 = ins["w2T6"]      # [768, 36] bf16  (chunks 0..5 of W2^T)
        w2Tt = ins["w2Tt"]      # [3, 36] bf16    (rows 768, 769, b2)
        vecsT = ins["vecsT"]    # [768, 504] bf16  (both batches, all j)
        myvT = ins["myvT"]      # [768, nslot] bf16 (per-core slot rows)
        maskb = ins["maskb"]    # [1, nslot*384] bf16: the 0/1 mask row
        maskrep = ins["maskrep"]  # [nslot*252] bf16 flat mask (s_out layout)
        e2f = ins["e2f"]        # [128, 2] f32: col b = 1 if this core owns row s_b
        outd = outs["out"]      # [36, nslot*252] bf16
        lseo = outs["lse"]      # [36, 2] f32

        fp = ctx.enter_context(tc.tile_pool(name="fp", bufs=1))
        prep_ps = ctx.enter_context(tc.tile_pool(name="prep_ps", bufs=2, space="PSUM"))
        main_ps = ctx.enter_context(tc.tile_pool(name="main_ps", bufs=4, space="PSUM"))
        hp = [ctx.enter_context(tc.tile_pool(name=f"h{c}", bufs=4)) for c in range(7)]
        esp = ctx.enter_context(tc.tile_pool(name="esp", bufs=2))
        dram = ctx.enter_context(tc.tile_pool(name="dram", bufs=1, space="DRAM"))

        # ---- persistent SBUF ----
        s_w1i = [fp.tile([128, MLP], BF16, tag=f"w1i{k}", name=f"w1i{k}") for k in range(KC)]
        s_w1j = [fp.tile([128, MLP], BF16, tag=f"w1j{k}", name=f"w1j{k}") for k in range(KC)]
        s_vT = [fp.tile([128, 2 * L], BF16, tag=f"vT{k}", name=f"vT{k}") for k in range(KC)]
        s_myv = [fp.tile([128, nslot], BF16, tag=f"myv{k}", name=f"myv{k}") for k in range(KC)]
        s_w2 = fp.tile([128, 6 * NLAB], BF16)      # W2T chunks 0..5 side by side
        s_w2t4 = fp.tile([3, NLAB], BF16)          # [W2T768, W2T769, b2]
        s_w1c = fp.tile([128, 7], FP32)            # w1c per-chunk columns
        s_b1 = fp.tile([128, 7], FP32)
        s_b1w = fp.tile([128, 7], FP32)
        s_e2f = fp.tile([128, 2], FP32)
        CS = [128] * KC + [2]                      # chunk sizes of 770
        s_bj0 = [fp.tile([128, B * 512], BF16, tag=f"bj0_{c}", name=f"bj0_{c}") for c in range(7)]
        s_bje = [fp.tile([128, B * 512], BF16, tag=f"bje_{c}", name=f"bje_{c}") for c in range(7)]
        s_ai = [fp.tile([128, nslot], FP32, tag=f"ai{c}", name=f"ai{c}") for c in range(7)]
        s_aiE2 = [fp.tile([128, B], FP32, tag=f"aiE2{c}", name=f"aiE2{c}") for c in range(7)]
        s_mrep = fp.tile([NLAB, nslot * L], BF16)  # mask replicated over labels
        s_acc = fp.tile([NLAB, ntile], FP32)
        s_out = handles["s_out"]
        s_x = handles["s_x"]
        s_g = handles["s_g"]
        s_nlse = handles["s_nlse"]

        # ---- load constants (spread across HWDGE queues) ----
        dmae = [nc.sync, nc.scalar]
        for k in range(KC):
            dmae[k % 2].dma_start(out=s_w1i[k], in_=w1iT[128 * k:128 * (k + 1), :])
            dmae[(k + 1) % 2].dma_start(out=s_w1j[k],
                                        in_=w1jT[128 * k:128 * (k + 1), :])
            dmae[k % 2].dma_start(out=s_vT[k],
                                  in_=vecsT[128 * k:128 * (k + 1), :])
            dmae[(k + 1) % 2].dma_start(out=s_myv[k],
                                        in_=myvT[128 * k:128 * (k + 1), :])
            dmae[k % 2].dma_start(out=s_w2[:, NLAB * k:NLAB * (k + 1)],
                                  in_=w2T6[128 * k:128 * (k + 1), :])
        nc.sync.dma_start(out=s_w2t4, in_=w2Tt)
        nc.vector.memset(s_w1c, 0.0)
        nc.vector.memset(s_b1, 0.0)
        w1c2 = w1c[0:HID].rearrange("(c p) -> p c", p=128)
        nc.sync.dma_start(out=s_w1c[:, 0:KC], in_=w1c2)
        nc.sync.dma_start(out=s_w1c[0:2, KC:7], in_=w1c[HID:MLP].rearrange("(p o) -> p o", o=1))
        b12 = b1[0:HID].rearrange("(c p) -> p c", p=128)
        nc.sync.dma_start(out=s_b1[:, 0:KC], in_=b12)
        nc.sync.dma_start(out=s_b1[0:2, KC:7], in_=b1[HID:MLP].rearrange("(p o) -> p o", o=1))
        nc.sync.dma_start(out=s_e2f, in_=e2f)
        nc.vector.tensor_tensor(out=s_b1w, in0=s_b1, in1=s_w1c, op=ALU.add)
        # replicated mask (one broadcast DMA per quarter to spread queues)
        Q = nslot * L // 4
        for q in range(4):
            src = maskrep[q * Q:(q + 1) * Q]
            nc.gpsimd.dma_start(
                out=s_mrep[:, q * Q:(q + 1) * Q],
                in_=bass.AP(tensor=src.tensor, offset=src.offset,
                            ap=[[0, NLAB], [1, Q]]))

        # ---- prep: AjT -> Bj0/BjE, AiT ----
        for c in range(7):
            cs = CS[c]
            mlo = 128 * c
            psA = prep_ps.tile([128, 2 * L], FP32, tag="psA", name=f"psA{c}")
            for k in range(KC):
                nc.tensor.matmul(psA[:cs, :], s_w1j[k][:, mlo:mlo + cs], s_vT[k],
                                 start=(k == 0), stop=(k == KC - 1))
            bje = s_bje[c]
            for b in range(B):
                e = spans[b][1]
                nc.vector.tensor_scalar(
                    out=s_bj0[c][:cs, 512 * b:512 * b + L],
                    in0=psA[:cs, L * b:L * (b + 1)],
                    scalar1=s_b1[:cs, c:c + 1], scalar2=None, op0=ALU.add)
                nc.gpsimd.memset(s_bj0[c][:cs, 512 * b + L:512 * (b + 1)], 0.0)
                nc.vector.tensor_scalar(
                    out=bje[:cs, 512 * b:512 * b + e + 1],
                    in0=psA[:cs, L * b:L * b + e + 1],
                    scalar1=s_b1w[:cs, c:c + 1], scalar2=None, op0=ALU.add)
                if e + 1 < L:
                    nc.vector.tensor_scalar(
                        out=bje[:cs, 512 * b + e + 1:512 * b + L],
                        in0=psA[:cs, L * b + e + 1:L * (b + 1)],
                        scalar1=s_b1[:cs, c:c + 1], scalar2=None, op0=ALU.add)
                nc.vector.memset(bje[:cs, 512 * b + L:512 * (b + 1)], 0.0)
            psI = prep_ps.tile([128, nslot], FP32, tag="psI", name=f"psI{c}")
            for k in range(KC):
                nc.tensor.matmul(psI[:cs, :], s_w1i[k][:, mlo:mlo + cs], s_myv[k],
                                 start=(k == 0), stop=(k == KC - 1))
            nc.vector.tensor_copy(out=s_ai[c][:cs, :], in_=psI[:cs, :])
            for b in range(B):
                p_e2 = next(sg for sg in segs
                            if sg["kind"] == "in" and sg["b"] == b)["start"]
                nc.vector.tensor_tensor(
                    out=s_aiE2[c][:cs, b:b + 1],
                    in0=s_w1c[:cs, c:c + 1], in1=s_e2f[:cs, b:b + 1],
                    op=ALU.mult)
                nc.vector.tensor_tensor(
                    out=s_aiE2[c][:cs, b:b + 1],
                    in0=s_aiE2[c][:cs, b:b + 1], in1=s_ai[c][:cs, p_e2:p_e2 + 1],
                    op=ALU.add)

        nc.vector.memset(s_x[0], 0.0)

        pid = {}

        def eng_pid(eng):
            if eng not in pid:
                pid[eng] = eng.partition_id()
            return pid[eng]

        def ts_relu(eng, out, in0, sc):
            if eng is nc.scalar:
                nc.scalar.activation(out, in0, AF.Relu, bias=sc, scale=1.0)
            else:
                eng.tensor_scalar(out=out, in0=in0, scalar1=sc, scalar2=0.0,
                                  op0=ALU.add, op1=ALU.max)

        # ---- main loop over 2-slot tiles ----
        for t in range(ntile):
            hts = [hp[c].tile([128 if c < 6 else 3, HW], BF16, tag=f"ht{c}",
                              name=f"ht{c}_{t}") for c in range(7)]
            nc.sync.dma_start(out=hts[6][2:3, :], in_=maskb[:, HW * t:HW * (t + 1)])
            # greedy DVE/ACT balance for this tile
            dve_load = DVE_TT
            act_load = ACT_EXP
            emits = []                # (which, args...)
            for c in range(7):
                cs = CS[c]
                for sl in range(2):
                    p = 2 * t + sl
                    b = slot_batch[p]
                    sg = slot_seg[p]
                    o_t, o_base = hts[c], SLOTW * sl
                    # segA
                    ca, cb = DVE_SEGA, ACT_SEGA
                    if sg["kind"] == "in":
                        kk = p - sg["start"]
                        Wd = max(1, sg["e"] - sg["s"] - NC * kk + 1)
                        act_load += act_segb_cost(Wd)
                        emits.append(("B", c, cs, o_t, o_base, p, b, sg, kk, Wd))
                    if dve_load + ca <= act_load + cb:
                        dve_load += ca
                        emits.append(("A", nc.vector, c, cs, o_t, o_base, p, b))
                    else:
                        act_load += cb
                        emits.append(("A", nc.scalar, c, cs, o_t, o_base, p, b))
                    if sg["kind"] == "in" and p == sg["start"]:
                        emits.append(("E", c, cs, o_t, o_base, b, sg))
            for em in emits:
                if em[0] == "A":
                    _, eng, c, cs, o_t, o_base, p, b = em
                    ts_relu(eng, o_t[:cs, o_base:o_base + 256],
                            s_bj0[c][:cs, 512 * b:512 * b + 256],
                            s_ai[c][:cs, p:p + 1])
                elif em[0] == "B":
                    _, c, cs, o_t, o_base, p, b, sg, kk, Wd = em
                    ioff = sg["s"] + NC * kk + eng_pid(nc.scalar)
                    ts_relu(nc.scalar, o_t[:cs, bass.ds(o_base + ioff, Wd)],
                            s_bje[c][:cs, bass.ds(512 * b + ioff, Wd)],
                            s_ai[c][:cs, p:p + 1])
                else:
                    _, c, cs, o_t, o_base, b, sg = em
                    e = sg["e"]
                    ts_relu(nc.vector, o_t[:cs, o_base + e:o_base + e + 1],
                            s_bje[c][:cs, 512 * b + e:512 * b + e + 1],
                            s_aiE2[c][:cs, b:b + 1])

            # matmul: psum[36, 504] over 6 full chunks + tail (h768/769, m*b2)
            ps = main_ps.tile([NLAB, 2 * L], FP32, tag="ps", name=f"ps{t}")
            rhs6 = [hts[c][:, :].rearrange("p (s w) -> p s w", w=SLOTW)
                    [:, :, 0:L] for c in range(6)]
            for c in range(6):
                nc.tensor.matmul(ps, s_w2[:, NLAB * c:NLAB * (c + 1)], rhs6[c],
                                 start=(c == 0), stop=False)
            rhs_t = hts[6][:, :].rearrange("p (s w) -> p s w", w=SLOTW)[:, :, 0:L]
            nc.tensor.matmul(ps, s_w2t4, rhs_t, start=False, stop=True)

            # masked logits -> bf16 out buffer
            nc.vector.tensor_tensor(out=s_out[:, 2 * L * t:2 * L * (t + 1)],
                                    in0=ps, in1=s_mrep[:, 2 * L * t:2 * L * (t + 1)],
                                    op=ALU.mult)
            # exp + per-tile softmax sum (invalid pairs contribute exp(0)=1,
            # exactly matching the reference where masked logits are 0)
            esc = esp.tile([NLAB, 2 * L], BF16, tag="esc", name=f"esc{t}")
            nc.scalar.activation(esc, s_out[:, 2 * L * t:2 * L * (t + 1)],
                                 AF.Exp, accum_out=s_acc[:, t:t + 1])

        # ---- LSE: tile sums + cross-core exchange + log ----
        for b in range(B):
            rs = btiles[b]
            runs = []
            st = rs[0]
            for a, bb in zip(rs, rs[1:] + [None]):
                if bb != a + 1:
                    runs.append((st, a))
                    st = bb
            acc_parts = []
            for (u0, u1) in runs:
                tmp = fp.tile([NLAB, 1], FP32, tag=f"tr{b}_{u0}", name=f"tr{b}_{u0}")
                nc.vector.tensor_reduce(out=tmp, in_=s_acc[:, u0:u1 + 1],
                                        axis=mybir.AxisListType.X, op=ALU.add)
                acc_parts.append(tmp)
            for tmp in acc_parts:
                nc.vector.tensor_tensor(out=s_x[0][0:NLAB, b:b + 1],
                                        in0=s_x[0][0:NLAB, b:b + 1], in1=tmp,
                                        op=ALU.add)

        if use_cc:
            cc_in = dram.tile([128, B], FP32, name="cc_in")
            cc_out = nc.dram_tensor("cc_out", [128, B], FP32, kind="Internal",
                                    addr_space="Shared").ap()
            nc.sync.dma_start(out=cc_in, in_=s_x[0])
            nc.gpsimd.collective_compute(
                "AllReduce", ALU.add, replica_groups=[list(range(NC))],
                ins=[cc_in[:]], outs=[cc_out], cc_dim="Partition")
            nc.sync.dma_start(out=s_x[3], in_=cc_out)

    return kern


def tail_groups(slot_batch, ntile):
    groups = []
    g0 = 0
    while g0 < ntile:
        if g0 + 1 < ntile and slot_batch[2 * (g0 + 1)] == slot_batch[2 * g0]:
            groups.append((g0, 2))
            g0 += 2
        else:
            groups.append((g0, 1))
            g0 += 1
    return groups


def build_post(nc, H, outd, lseo, slot_batch, ntile, use_cc):
    """Raw-bass epilogue: butterfly all-reduce of the exp sums, -log(lse),
    tail adds and output DMAs.  Engine FIFO order + explicit semaphores."""
    gp, dv, ac, sy = nc.gpsimd, nc.vector, nc.scalar, nc.sync
    s_x, s_g, s_out, s_nlse = H["s_x"], H["s_g"], H["s_out"], H["s_nlse"]

    nc.all_engine_barrier()
    sems = H["sems"]
    nsem, dsem, asem, osem, xsem = (sems["nsem"], sems["dsem"], sems["asem"],
                                    sems["osem"], sems["xsem"])
    gsem = sems["gsem"]
    if not use_cc:
        lsem, psem = sems["lsem"], sems["psem"]
        rsem = sems["rsem"]
        gp.bir_kernel_barrier_wait([list(range(NC))])
        for r in range(3):
            if r > 0:
                gp.wait_ge(xsem, r)
            prep = gp.remote_dma_broadcast(
                out_ap=s_g[r][:, :], in_ap=s_x[r][:, :],
                remote_sem=rsem[r], local_sem=lsem,
                rdests=[(0, 1 << r) if k == (1 << r) else None
                        for k in range(8)])
            prep.then_inc(psem, 1)
            gp.wait_ge(psem, r + 1)
            gp.trigger_dma(count=1)
            dv.wait_ge(rsem[r], 2)
            dv.tensor_tensor(out=s_x[r + 1], in0=s_x[r], in1=s_g[r],
                             op=ALU.add).then_inc(xsem, 1)
    else:
        dv.tensor_copy(out=s_x[3], in_=s_x[3]).then_inc(xsem, 3)

    ac.wait_ge(xsem, 3)
    ac.activation(s_nlse, s_x[3][0:NLAB, :], AF.Ln).then_inc(nsem, 1)
    dv.wait_ge(nsem, 1)
    dv.tensor_scalar(out=s_nlse, in0=s_nlse, scalar1=-1.0,
                     scalar2=None, op0=ALU.mult).then_inc(nsem, 1)
    dv.wait_ge(nsem, 2)
    ac.wait_ge(nsem, 2)
    sy.wait_ge(xsem, 3)
    sy.dma_start(out=lseo, in_=s_x[3][0:NLAB, :]).then_inc(osem, 16)

    groups = tail_groups(slot_batch, ntile)
    kd = ka = nout = 0
    for gi, (t0, gn) in enumerate(groups):
        b = slot_batch[2 * t0]
        seg = s_out[:, 2 * L * t0:2 * L * (t0 + gn)]
        oseg = outd[:, 2 * L * t0:2 * L * (t0 + gn)]
        if gi % 2 == 0:
            dv.tensor_scalar(out=seg, in0=seg, scalar1=s_nlse[:, b:b + 1],
                             scalar2=None, op0=ALU.add).then_inc(dsem, 1)
            kd += 1
            sy.wait_ge(dsem, kd)
            sy.dma_start(out=oseg, in_=seg).then_inc(osem, 16)
        else:
            ac.activation(seg, seg, AF.Identity,
                          bias=s_nlse[:, b:b + 1], scale=1.0).then_inc(asem, 1)
            ka += 1
            gp.wait_ge(asem, ka)
            gp.dma_start(out=oseg, in_=seg).then_inc(gsem, 16)
        nout += 1
    gp.wait_ge(osem, 16 * (nout - ka + 1))
    gp.wait_ge(gsem, 16 * ka)


def kernel(**inputs) -> np.ndarray:
    hidden = np.asarray(inputs["hidden"], dtype=np.float32)
    pred_spans = np.asarray(inputs["pred_spans"]).astype(np.int64)
    span_mask = np.asarray(inputs["span_mask"]).astype(np.int32)
    W1 = np.asarray(inputs["W1"], dtype=np.float32)
    b1 = np.asarray(inputs["b1"], dtype=np.float32)
    W2 = np.asarray(inputs["W2"], dtype=np.float32)
    b2 = np.asarray(inputs["b2"], dtype=np.float32)

    spans = [(int(pred_spans[b, 0]), int(pred_spans[b, 1])) for b in range(B)]
    segs, nslot = plan_slots(spans)
    ntile = nslot // 2

    vecs = hidden[:, 1:L + 1, :]                       # [B, L, 768]
    vecsT = np.concatenate([vecs[0].T, vecs[1].T], axis=1)   # [768, 504]
    W1T = W1.T                                          # [1537, 770]
    w1iT = np.ascontiguousarray(W1T[0:HID]).astype(np.float32)
    w1jT = np.ascontiguousarray(W1T[HID:2 * HID]).astype(np.float32)
    w1c = np.ascontiguousarray(W1T[2 * HID]).astype(np.float32)
    W2T = np.ascontiguousarray(W2.T)                    # [770, 36]
    w2T6 = W2T[0:HID]
    w2Tt = np.stack([W2T[768], W2T[769], b2], axis=0)

    maskf_full = span_mask.astype(np.float32).clip(0, 1)

    in_maps = []
    slot_maps = []
    for c in range(NC):
        sm = slot_map_for_core(segs, nslot, c)
        slot_maps.append(sm)
        myv = np.zeros((HID, nslot), np.float32)
        maskf = np.zeros((nslot, L), np.float32)
        flags = np.zeros((nslot, 2), np.float32)
        for p, ent in enumerate(sm):
            if ent is None:
                continue
            b, r = ent
            myv[:, p] = vecs[b, r]
            maskf[p] = maskf_full[r]
            flags[p, b] = 1.0
        maskb = np.zeros((1, nslot * 384), np.float32)
        for t in range(ntile):
            for sl in range(2):
                p = 2 * t + sl
                o = 768 * t + 384 * sl
                maskb[0, o:o + L] = maskf[p]
        maskrep = maskf.reshape(-1)                     # [nslot*252]
        e2f = np.zeros((128, 2), np.float32)
        for b in range(B):
            if c == 0:
                e2f[:, b] = 1.0      # row s_b lives on core 0 (slot seg start)
        in_maps.append({
            "w1iT": w1iT, "w1jT": w1jT, "w1c": w1c, "b1": b1,
            "w2T6": w2T6, "w2Tt": w2Tt,
            "vecsT": vecsT, "myvT": myv,
            "maskb": maskb, "maskrep": maskrep, "e2f": e2f,
        })

    # ---- build program ----
    nc = bacc.Bacc("TRN2", target_bir_lowering=False, debug=False,
                   enable_asserts=False, num_devices=NC)

    def mk(name, arr, dt):
        return nc.dram_tensor(name, list(arr.shape), dt, kind="ExternalInput").ap()

    ml_bf = lambda n, a: mk(n, a, BF16)
    ml_f32 = lambda n, a: mk(n, a, FP32)
    ex = in_maps[0]
    ins_aps = {
        "w1iT": ml_bf("w1iT", ex["w1iT"]), "w1jT": ml_bf("w1jT", ex["w1jT"]),
        "w1c": ml_f32("w1c", ex["w1c"]), "b1": ml_f32("b1", ex["b1"]),
        "w2T6": ml_bf("w2T6", ex["w2T6"]), "w2Tt": ml_bf("w2Tt", ex["w2Tt"]),
        "vecsT": ml_bf("vecsT", ex["vecsT"]), "myvT": ml_bf("myvT", ex["myvT"]),
        "maskb": ml_bf("maskb", ex["maskb"]),
        "maskrep": ml_bf("maskrep", ex["maskrep"]),
        "e2f": ml_f32("e2f", ex["e2f"]),
    }
    outs_aps = {
        "out": nc.dram_tensor("out", [NLAB, nslot * L], BF16,
                              kind="ExternalOutput").ap(),
        "lse": nc.dram_tensor("lse", [NLAB, 2], FP32,
                              kind="ExternalOutput").ap(),
    }

    use_cc = bool(int(os.environ.get("BK_CC", "1")))
    handles = {
        "s_out": nc.alloc_sbuf_tensor("s_out", [NLAB, nslot * L], BF16).ap(),
        "s_x": [nc.alloc_sbuf_tensor(f"s_x{r}", [128, B], FP32).ap()
                for r in range(4)],
        "s_g": [nc.alloc_sbuf_tensor(f"s_g{r}", [128, B], FP32).ap()
                for r in range(3)],
        "s_nlse": nc.alloc_sbuf_tensor("s_nlse", [NLAB, B], FP32).ap(),
        "sems": {
            "nsem": nc.alloc_semaphore("nsem"),
            "dsem": nc.alloc_semaphore("dsem"),
            "asem": nc.alloc_semaphore("asem"),
            "osem": nc.alloc_semaphore("osem"),
            "xsem": nc.alloc_semaphore("xsem"),
            "lsem": nc.alloc_semaphore("rdma_lsem"),
            "psem": nc.alloc_semaphore("rdma_psem"),
            "gsem": nc.alloc_semaphore("gsem"),
            "rsem": [nc.alloc_semaphore(f"rdma_rsem{r}") for r in range(3)],
        },
    }
    kern = build_kernel(spans, segs, nslot, use_cc, handles)
    with tile.TileContext(nc) as t:
        kern(t, outs_aps, ins_aps)
    slot_batch = []
    for sg in segs:
        for k in range(sg["nslots"]):
            slot_batch.append(sg["b"])
    build_post(nc, handles, outs_aps["out"], outs_aps["lse"],
               slot_batch, ntile, use_cc)
    nc.compile()

    def cast_maps(m):
        out = {}
        for k, v in m.items():
            dt = ins_aps[k].dtype
            if dt == BF16:
                out[k] = v.astype(mybir.dt.np(BF16))
            else:
                out[k] = v.astype(np.float32)
        return out

    in_maps_c = [cast_maps(m) for m in in_maps]

    if os.environ.get("BK_BUILD_ONLY"):
        print("BUILD OK")
        return np.zeros((B, NLAB, L * L), np.float32)

    if os.environ.get("BK_SIM"):
        import concourse.bass_interp as bi
        from concourse.bass_interp import MultiCoreSim

        # fake-nrt container: no real device maps; 8 cores = device 0, nc 0-7
        import concourse.libnrt as lnrt
        lnrt.get_trn2_nc_mapping = lambda: {(0, i): i for i in range(8)}
        _rmap = lambda: {0: 0}
        lnrt.get_device_id_to_routing_id_mapping = _rmap
        bi.pnc_id_to_device_and_real_nc_index = lambda cid: (0, cid % 8)
        bi.get_device_id_to_routing_id_mapping = _rmap
        bi.nc_to_real_nc = lambda dev, nc: nc

        nc.m.detect_race_conditions = False
        sim = MultiCoreSim(nc, num_cores=NC, require_finite=False,
                           require_nnan=False)
        for c, cs in sim.cores.items():
            for name, arr in in_maps_c[c].items():
                cs.tensor(name)[:] = arr
            if nc.partition_id_tensor is not None:
                cs.tensor(nc.partition_id_tensor.name)[:] = np.array(
                    [[c]], dtype=np.uint32)
        sim.simulate(check_with_hw=False)

        class _R:
            results = [{"out": np.asarray(sim.cores[c].tensor("out")),
                        "lse": np.asarray(sim.cores[c].tensor("lse"))}
                       for c in range(NC)]
        res = _R()
    else:
        trace = bool(int(os.environ.get("BK_TRACE", "0")))
        res = run_bass_kernel_spmd(nc, in_maps_c, core_ids=list(range(NC)),
                                   trace=trace)
        if trace and res.exec_time_ns is not None:
            print(f"HW exec time: {res.exec_time_ns} ns")

    if os.environ.get("BK_DBG_LSE"):
        for c in range(NC):
            print(f"core{c} lse[0:3,:]:", np.asarray(res.results[c]["lse"])[0:3].tolist())
    # ---- unshard ----
    out_full = np.zeros((B, NLAB, L * L), np.float32)
    for c in range(NC):
        oc = np.asarray(res.results[c]["out"]).astype(np.float32)
        for p, ent in enumerate(slot_maps[c]):
            if ent is None:
                continue
            b, r = ent
            out_full[b, :, L * r:L * (r + 1)] = oc[:, L * p:L * (p + 1)]
    return out_full
